# revision 14
# baseline (speedup 1.0000x reference)
"""2-layer GCN (2 edge types + self loop) on 8 TRN2 NeuronCores.

Sharding: nodes split contiguously across 8 cores (6250/core, padded to
6272 = 49 windows x 128 rows); edge lists partitioned by destination
owner, sorted by (dst window, src half, dst slot); [128,128] weights
replicated. Transformed feature tables (m = h @ W, bf16) are AllGathered
each layer; segment-sum runs as per-chunk selection-matrix matmuls
accumulating in PSUM with the self-loop matmul folded in and bias+ReLU
fused into the PSUM->SBUF copy on the scalar engine.

Gathers are per (etype, window, half) pieces (~1152 rows each) issued as
SWDGE prepare_only descriptor preps rotating over 4 DMA queues, with the
4 triggers following the 4 preps of each window: descriptor generation
for window w+1 overlaps the HBM drains of window w, and 4 queues keep
more HBM reads in flight. Mask builds run in DVE 1x mode (odd width +
2-byte misaligned tiles) so they never take the shared GPSIMD/DVE SBUF
port pair, which would starve SWDGE descriptor generation.
"""
import sys
import math

sys.path.insert(0, "/opt/trn_rl_repo")

import numpy as np
import ml_dtypes

import concourse.bass as bass  # noqa: F401
import concourse.bacc as bacc
import concourse.mybir as mybir
import concourse.tile as tile
from concourse.bass_utils import run_bass_kernel_spmd

N = 50000
D = 128
NCORES = 8
LOCAL = 6250          # real rows per core
SHARD = 6272          # padded rows per core (49 windows of 128)
NW = 49               # dst windows per core
VN = SHARD * NCORES   # 50176 virtual node rows
HALFR = VN // 2       # 25088, int16-addressable half of the gathered table
NQ = 4                # SWDGE queues

F32 = mybir.dt.float32
BF16 = mybir.dt.bfloat16
I16 = mybir.dt.int16
BF = ml_dtypes.bfloat16

_compiled = {}
last_results = None  # set when BASS_KERNEL_TRACE=1; holds BassKernelResults


def _prep_etype(src, dst):
    """Per-edge-type host prep. Returns (K[w][h] chunk table, chunk
    offsets, chunk count, per-core wrapped idx / dst_rel / v arrays in
    canonical chunk order: for w, for h, for k)."""
    ar = np.arange(N, dtype=np.int64)
    vmap = (ar // LOCAL) * SHARD + ar % LOCAL
    deg = np.bincount(dst, minlength=N).astype(np.float32)
    v_edge = (1.0 / np.maximum(deg, 1.0))[dst].astype(np.float32)
    dvid = vmap[dst]
    r = dvid // SHARD
    w = (dvid % SHARD) // 128
    wloc = (dvid % 128).astype(np.float32)
    svid = vmap[src]
    h = svid // HALFR
    i16 = (svid % HALFR).astype(np.int16)

    key = (r * NW + w) * 2 + h
    order = np.argsort(key, kind="stable")
    counts = np.bincount(key, minlength=NCORES * NW * 2).reshape(NCORES, NW, 2)
    flat = counts.reshape(-1)
    fs = np.concatenate([[0], np.cumsum(flat)[:-1]])
    starts = fs.reshape(NCORES, NW, 2)

    K = np.maximum(1, (counts.max(axis=0) + 127) // 128)  # [NW, 2]
    assert K.max() * 128 <= 2048, f"gather piece too big: K={K.max()}"

    # canonical chunk order: w-major, then half, then k
    chunk_off = {}
    nch = 0
    for ww in range(NW):
        for hh in (0, 1):
            chunk_off[(ww, hh)] = nch
            nch += int(K[ww, hh])

    i16_s = i16[order]
    wloc_s = wloc[order]
    v_s = v_edge[order]

    idx_all, dst_all, v_all = [], [], []
    for rr in range(NCORES):
        idx_pad = np.zeros(nch * 128, np.int16)
        dst_pad = np.full(nch * 128, -1.0, np.float32)
        v_pad = np.ones(nch * 128, np.float32)
        for ww in range(NW):
            for hh in (0, 1):
                s0 = starts[rr, ww, hh]
                c = counts[rr, ww, hh]
                o = chunk_off[(ww, hh)] * 128
                idx_pad[o : o + c] = i16_s[s0 : s0 + c]
                dst_pad[o : o + c] = wloc_s[s0 : s0 + c]
                v_pad[o : o + c] = v_s[s0 : s0 + c]
        wrapped = np.tile(idx_pad.reshape(-1, 16).T, (8, 1))  # [128, nch*8]
        idx_all.append(np.ascontiguousarray(wrapped))
        dst_all.append(np.ascontiguousarray(dst_pad.reshape(nch, 128).T))
        v_all.append(np.ascontiguousarray(v_pad.reshape(nch, 128).T))
    return K, chunk_off, nch, idx_all, dst_all, v_all


def _build(K_a, off_a, nch_a, K_b, off_b, nch_b):
    nc = bacc.Bacc("TRN2", target_bir_lowering=False, debug=False,
                   num_swdge_queues=NQ, dynamic_dma_scratch_size=81920)

    xT_in = nc.dram_tensor("xT", [128, SHARD], BF16, kind="ExternalInput")
    w_names = ["W_proj", "W1_a", "W1_b", "loop1", "W2_a", "W2_b", "loop2"]
    w_in = {n: nc.dram_tensor(n, [128, 128], BF16, kind="ExternalInput") for n in w_names}
    b_names = ["bias_proj", "bias1", "bias2"]
    b_in = {n: nc.dram_tensor(n, [128, 1], F32, kind="ExternalInput") for n in b_names}
    iota_in = nc.dram_tensor("iota", [128, 130], BF16, kind="ExternalInput")
    idx_in = [
        nc.dram_tensor("idx_a", [128, nch_a * 8], I16, kind="ExternalInput"),
        nc.dram_tensor("idx_b", [128, nch_b * 8], I16, kind="ExternalInput"),
    ]
    dst_in = [
        nc.dram_tensor("dst_a", [128, nch_a], F32, kind="ExternalInput"),
        nc.dram_tensor("dst_b", [128, nch_b], F32, kind="ExternalInput"),
    ]
    v_in = [
        nc.dram_tensor("v_a", [128, nch_a], F32, kind="ExternalInput"),
        nc.dram_tensor("v_b", [128, nch_b], F32, kind="ExternalInput"),
    ]
    out = nc.dram_tensor("out", [128, SHARD], F32, kind="ExternalOutput")

    Ks = [K_a, K_b]
    offs = [off_a, off_b]
    nchs = [nch_a, nch_b]

    with tile.TileContext(nc) as tc:
        with (
            tc.tile_pool(name="sbuf", bufs=1) as sb,
            tc.tile_pool(name="psum", bufs=2, space="PSUM") as ps,
            tc.tile_pool(name="dram", bufs=1, space="DRAM") as dr,
        ):
            # ---- constants / persistent buffers
            w_sb = {}
            for n in w_names:
                w_sb[n] = sb.tile([128, 128], BF16, tag=f"w_{n}", name=f"w_{n}")
                nc.sync.dma_start(out=w_sb[n][:], in_=w_in[n][:])
            b_sb = {}
            for n in b_names:
                b_sb[n] = sb.tile([128, 1], F32, tag=f"b_{n}", name=f"b_{n}")
                nc.sync.dma_start(out=b_sb[n][:], in_=b_in[n][:])
            iota_sb = sb.tile([128, 130], BF16, tag="iota")
            nc.sync.dma_start(out=iota_sb[:], in_=iota_in[:])

            xT = sb.tile([128, SHARD], BF16, tag="xT")
            nc.sync.dma_start(out=xT[:], in_=xT_in[:])
            dst_sb = []
            v_sb = []
            idx_sb = []
            for t in (0, 1):
                d = sb.tile([128, nchs[t]], F32, tag=f"dst{t}", name=f"dst{t}")
                nc.sync.dma_start(out=d[:], in_=dst_in[t][:])
                dst_sb.append(d)
                vv = sb.tile([128, nchs[t]], F32, tag=f"v{t}", name=f"v{t}")
                nc.sync.dma_start(out=vv[:], in_=v_in[t][:])
                v_sb.append(vv)
                ii = sb.tile([128, nchs[t] * 8], I16, tag=f"idx{t}", name=f"idx{t}")
                nc.sync.dma_start(out=ii[:], in_=idx_in[t][:])
                idx_sb.append(ii)

            hT = sb.tile([128, SHARD], BF16, tag="hT")
            h1T = sb.tile([128, SHARD], BF16, tag="h1T")
            h2T = sb.tile([128, SHARD], F32, tag="h2T")

            # AllGather buffers (per layer, per etype)
            m_in = [[dr.tile([SHARD, 128], BF16, tag=f"mi{l}{t}", name=f"mi{l}{t}") for t in (0, 1)] for l in (0, 1)]
            m_out = [[dr.tile([VN, 128], BF16, tag=f"mo{l}{t}", name=f"mo{l}{t}", addr_space="Shared") for t in (0, 1)] for l in (0, 1)]

            qsem = [nc.alloc_semaphore(f"qsem{q}") for q in range(NQ)]
            qcnt = [0] * NQ

            def col_chunks(total, step):
                o = 0
                while o < total:
                    yield o, min(step, total - o)
                    o += step

            # ---- phase P: hT = (x @ W_proj + b_proj)^T
            for o, n in col_chunks(SHARD, 512):
                p = ps.tile([128, 512], F32, tag="pdense")
                nc.tensor.matmul(p[:, :n], lhsT=w_sb["W_proj"][:], rhs=xT[:, o : o + n],
                                 start=True, stop=True)
                nc.vector.tensor_scalar_add(hT[:, o : o + n], p[:, :n], b_sb["bias_proj"][:, :1])

            # ---- layers
            for l in (0, 1):
                src_hT = hT if l == 0 else h1T
                dstT = h1T if l == 0 else h2T
                wa, wb, wl = (("W1_a", "W1_b", "loop1") if l == 0 else ("W2_a", "W2_b", "loop2"))
                bias = b_sb["bias1"] if l == 0 else b_sb["bias2"]

                # m tables (node-major, bf16) + AllGather
                for t, wn in ((0, wa), (1, wb)):
                    for w in range(NW):
                        pm = ps.tile([128, 512], F32, tag="pdense")
                        nc.tensor.matmul(pm[:, :128], lhsT=src_hT[:, w * 128 : (w + 1) * 128],
                                         rhs=w_sb[wn][:], start=True, stop=True)
                        ms = sb.tile([128, 128], BF16, tag="ms")
                        nc.scalar.activation(out=ms[:], in_=pm[:, :128],
                                             func=mybir.ActivationFunctionType.Copy)
                        nc.sync.dma_start(out=m_in[l][t][w * 128 : (w + 1) * 128, :], in_=ms[:])
                    nc.gpsimd.collective_compute(
                        "AllGather",
                        mybir.AluOpType.bypass,
                        replica_groups=[list(range(NCORES))],
                        ins=[m_in[l][t].opt()],
                        outs=[m_out[l][t].opt()],
                    )

                # aggregation: per window, 4 gather pieces on 4 queues
                for w in range(NW):
                    pieces = [(t, hh) for t in (0, 1) for hh in (0, 1)]
                    gb = {}
                    for q, (t, hh) in enumerate(pieces):
                        kk = int(Ks[t][w, hh])
                        ci0 = offs[t][(w, hh)]
                        gbuf = sb.tile([128, kk, 128], BF16, tag=f"gb{t}{hh}",
                                       name=f"gb{l}{t}{hh}{w}", bufs=3)
                        nc.gpsimd.dma_gather(
                            gbuf[:],
                            m_out[l][t][hh * HALFR : (hh + 1) * HALFR, :],
                            idx_sb[t][:, ci0 * 8 : (ci0 + kk) * 8],
                            kk * 128,
                            kk * 128,
                            128,
                            single_packet=False,
                            prepare_only=True,
                            sem=qsem[q],
                            queue_num=q,
                        )
                        gb[(t, hh)] = gbuf
                        qcnt[q] += 1
                    for q in range(NQ):
                        nc.gpsimd.trigger_dma(count=None, queue_num=q)
                    for q in range(NQ):
                        nc.tensor.wait_ge(qsem[q], 16 * qcnt[q])

                    pw = ps.tile([128, 128], F32, tag="pw", bufs=4)
                    first = True
                    for t, hh in pieces:
                        gbuf = gb[(t, hh)]
                        for k in range(int(Ks[t][w, hh])):
                            ci = offs[t][(w, hh)] + k
                            # odd width + 2B-misaligned tile keeps this in
                            # DVE 1x mode (no shared-port lock vs SWDGE)
                            s = sb.tile([128, 130], BF16, tag="s", bufs=8)
                            nc.vector.tensor_scalar(
                                out=s[:, 1:130],
                                in0=iota_sb[:, 1:130],
                                scalar1=dst_sb[t][:, ci : ci + 1],
                                scalar2=v_sb[t][:, ci : ci + 1],
                                op0=mybir.AluOpType.is_equal,
                                op1=mybir.AluOpType.mult,
                            )
                            nc.tensor.matmul(pw[:], lhsT=gbuf[:, k, :], rhs=s[:, 1:129],
                                             start=first, stop=False)
                            first = False
                    nc.tensor.matmul(pw[:], lhsT=w_sb[wl][:],
                                     rhs=src_hT[:, w * 128 : (w + 1) * 128],
                                     start=False, stop=True)
                    nc.scalar.activation(out=dstT[:, w * 128 : (w + 1) * 128], in_=pw[:],
                                         func=mybir.ActivationFunctionType.Relu,
                                         bias=bias[:, :1], scale=1.0)

            nc.sync.dma_start(out=out[:], in_=h2T[:])
    nc.compile()
    return nc


def kernel(**inputs):
    x = np.asarray(inputs["x"], np.float32)
    prep_a = _prep_etype(np.asarray(inputs["src_a"]), np.asarray(inputs["dst_a"]))
    prep_b = _prep_etype(np.asarray(inputs["src_b"]), np.asarray(inputs["dst_b"]))
    K_a, off_a, nch_a, idx_a, dst_a, v_a = prep_a
    K_b, off_b, nch_b, idx_b, dst_b, v_b = prep_b

    key = (nch_a, nch_b, K_a.tobytes(), K_b.tobytes())
    if key not in _compiled:
        _compiled[key] = _build(K_a, off_a, nch_a, K_b, off_b, nch_b)
    nc = _compiled[key]

    x_pad = np.zeros((NCORES, SHARD, D), np.float32)
    x_pad[:, :LOCAL] = x.reshape(NCORES, LOCAL, D)

    weights = {
        "W_proj": inputs["W_proj"], "W1_a": inputs["W1_a"], "W1_b": inputs["W1_b"],
        "loop1": inputs["loop1"], "W2_a": inputs["W2_a"], "W2_b": inputs["W2_b"],
        "loop2": inputs["loop2"],
    }
    w_np = {k: np.asarray(v, np.float32).astype(BF) for k, v in weights.items()}
    biases = {
        "bias_proj": np.asarray(inputs["b_proj"], np.float32).reshape(128, 1),
        "bias1": (np.asarray(inputs["b1_a"], np.float32)
                  + np.asarray(inputs["b1_b"], np.float32)).reshape(128, 1),
        "bias2": (np.asarray(inputs["b2_a"], np.float32)
                  + np.asarray(inputs["b2_b"], np.float32)).reshape(128, 1),
    }
    # iota[:, j] = j-1 (cols 1..129 hold 0..128); col 0 unused
    iota = np.tile((np.arange(130, dtype=np.float32) - 1.0).astype(BF), (128, 1))

    in_maps = []
    for c in range(NCORES):
        m = {
            "xT": np.ascontiguousarray(x_pad[c].T).astype(BF),
            "iota": iota,
            "idx_a": idx_a[c], "idx_b": idx_b[c],
            "dst_a": dst_a[c], "dst_b": dst_b[c],
            "v_a": v_a[c], "v_b": v_b[c],
        }
        m.update(w_np)
        m.update(biases)
        in_maps.append(m)

    import os
    global last_results
    if os.environ.get("BASS_KERNEL_TRACE"):
        res = run_bass_kernel_spmd(nc, in_maps, core_ids=list(range(NCORES)),
                                   trace=True)
        last_results = res
    else:
        res = run_bass_kernel_spmd(nc, in_maps, core_ids=list(range(NCORES)))
    full = np.concatenate(
        [np.asarray(res.results[c]["out"]).T[:LOCAL] for c in range(NCORES)], axis=0
    )
    return full.astype(np.float32)


# revision 16
# speedup vs baseline: 1.2014x; 1.2014x over previous
"""2-layer GCN (2 edge types + self loop) on 8 TRN2 NeuronCores.

Sharding: nodes split contiguously across 8 cores (6250/core, padded to
6272 = 49 windows x 128 rows); edge lists partitioned by destination
owner, sorted by (dst window, src half, dst slot); [128,128] weights
replicated. Transformed feature tables (m = h @ W, bf16) are AllGathered
each layer; segment-sum runs as per-chunk selection-matrix matmuls
accumulating in PSUM with the self-loop matmul folded in and bias+ReLU
fused into the PSUM->SBUF copy on the scalar engine.

Gathers are per (etype, window, half) pieces (~1152 rows each) issued as
SWDGE prepare_only descriptor preps rotating over 4 DMA queues, with the
4 triggers following the 4 preps of each window: descriptor generation
for window w+1 overlaps the HBM drains of window w, and 4 queues keep
more HBM reads in flight. Mask builds run in DVE 1x mode (odd width +
2-byte misaligned tiles) so they never take the shared GPSIMD/DVE SBUF
port pair, which would starve SWDGE descriptor generation.
"""
import sys
import math

sys.path.insert(0, "/opt/trn_rl_repo")

import numpy as np
import ml_dtypes

import concourse.bass as bass  # noqa: F401
import concourse.bacc as bacc
import concourse.mybir as mybir
import concourse.tile as tile
from concourse.bass_utils import run_bass_kernel_spmd

N = 50000
D = 128
NCORES = 8
LOCAL = 6250          # real rows per core
SHARD = 6272          # padded rows per core (49 windows of 128)
NW = 49               # dst windows per core
VN = SHARD * NCORES   # 50176 virtual node rows
HALFR = VN // 2       # 25088, int16-addressable half of the gathered table
NQ = 4                # SWDGE queues

F32 = mybir.dt.float32
BF16 = mybir.dt.bfloat16
I16 = mybir.dt.int16
BF = ml_dtypes.bfloat16

_compiled = {}
last_results = None  # set when BASS_KERNEL_TRACE=1; holds BassKernelResults


def _prep_etype(src, dst):
    """Per-edge-type host prep. Returns (K[w][h] chunk table, chunk
    offsets, chunk count, per-core wrapped idx / dst_rel / v arrays in
    canonical chunk order: for w, for h, for k)."""
    ar = np.arange(N, dtype=np.int64)
    vmap = (ar // LOCAL) * SHARD + ar % LOCAL
    deg = np.bincount(dst, minlength=N).astype(np.float32)
    v_edge = (1.0 / np.maximum(deg, 1.0))[dst].astype(np.float32)
    dvid = vmap[dst]
    r = dvid // SHARD
    w = (dvid % SHARD) // 128
    wloc = (dvid % 128).astype(np.float32)
    svid = vmap[src]
    h = svid // HALFR
    i16 = (svid % HALFR).astype(np.int16)

    key = (r * NW + w) * 2 + h
    order = np.argsort(key, kind="stable")
    counts = np.bincount(key, minlength=NCORES * NW * 2).reshape(NCORES, NW, 2)
    flat = counts.reshape(-1)
    fs = np.concatenate([[0], np.cumsum(flat)[:-1]])
    starts = fs.reshape(NCORES, NW, 2)

    K = np.maximum(1, (counts.max(axis=0) + 127) // 128)  # [NW, 2]
    assert K.max() * 128 <= 2048, f"gather piece too big: K={K.max()}"

    # canonical chunk order: w-major, then half, then k
    chunk_off = {}
    nch = 0
    for ww in range(NW):
        for hh in (0, 1):
            chunk_off[(ww, hh)] = nch
            nch += int(K[ww, hh])

    i16_s = i16[order]
    wloc_s = wloc[order]
    v_s = v_edge[order]

    idx_all, dst_all, v_all = [], [], []
    for rr in range(NCORES):
        idx_pad = np.zeros(nch * 128, np.int16)
        dst_pad = np.full(nch * 128, -1.0, np.float32)
        v_pad = np.ones(nch * 128, np.float32)
        for ww in range(NW):
            for hh in (0, 1):
                s0 = starts[rr, ww, hh]
                c = counts[rr, ww, hh]
                o = chunk_off[(ww, hh)] * 128
                idx_pad[o : o + c] = i16_s[s0 : s0 + c]
                dst_pad[o : o + c] = wloc_s[s0 : s0 + c]
                v_pad[o : o + c] = v_s[s0 : s0 + c]
        wrapped = np.tile(idx_pad.reshape(-1, 16).T, (8, 1))  # [128, nch*8]
        idx_all.append(np.ascontiguousarray(wrapped))
        dst_all.append(np.ascontiguousarray(dst_pad.reshape(nch, 128).T))
        v_all.append(np.ascontiguousarray(v_pad.reshape(nch, 128).T))
    return K, chunk_off, nch, idx_all, dst_all, v_all


def _build(K_a, off_a, nch_a, K_b, off_b, nch_b):
    nc = bacc.Bacc("TRN2", target_bir_lowering=False, debug=False,
                   num_swdge_queues=NQ, dynamic_dma_scratch_size=81920)

    xT_in = nc.dram_tensor("xT", [128, SHARD], BF16, kind="ExternalInput")
    w_names = ["W_proj", "W1_a", "W1_b", "loop1", "W2_a", "W2_b", "loop2"]
    w_in = {n: nc.dram_tensor(n, [128, 128], BF16, kind="ExternalInput") for n in w_names}
    b_names = ["bias_proj", "bias1", "bias2"]
    b_in = {n: nc.dram_tensor(n, [128, 1], F32, kind="ExternalInput") for n in b_names}
    iota_in = nc.dram_tensor("iota", [128, 130], BF16, kind="ExternalInput")
    idx_in = [
        nc.dram_tensor("idx_a", [128, nch_a * 8], I16, kind="ExternalInput"),
        nc.dram_tensor("idx_b", [128, nch_b * 8], I16, kind="ExternalInput"),
    ]
    dst_in = [
        nc.dram_tensor("dst_a", [128, nch_a], F32, kind="ExternalInput"),
        nc.dram_tensor("dst_b", [128, nch_b], F32, kind="ExternalInput"),
    ]
    v_in = [
        nc.dram_tensor("v_a", [128, nch_a], F32, kind="ExternalInput"),
        nc.dram_tensor("v_b", [128, nch_b], F32, kind="ExternalInput"),
    ]
    out = nc.dram_tensor("out", [128, SHARD], F32, kind="ExternalOutput")

    Ks = [K_a, K_b]
    offs = [off_a, off_b]
    nchs = [nch_a, nch_b]

    with tile.TileContext(nc) as tc:
        with (
            tc.tile_pool(name="sbuf", bufs=1) as sb,
            tc.tile_pool(name="psum", bufs=2, space="PSUM") as ps,
            tc.tile_pool(name="dram", bufs=1, space="DRAM") as dr,
        ):
            # ---- constants / persistent buffers
            w_sb = {}
            for n in w_names:
                w_sb[n] = sb.tile([128, 128], BF16, tag=f"w_{n}", name=f"w_{n}")
                nc.sync.dma_start(out=w_sb[n][:], in_=w_in[n][:])
            b_sb = {}
            for n in b_names:
                b_sb[n] = sb.tile([128, 1], F32, tag=f"b_{n}", name=f"b_{n}")
                nc.sync.dma_start(out=b_sb[n][:], in_=b_in[n][:])
            iota_sb = sb.tile([128, 130], BF16, tag="iota")
            nc.sync.dma_start(out=iota_sb[:], in_=iota_in[:])

            xT = sb.tile([128, SHARD], BF16, tag="xT")
            nc.sync.dma_start(out=xT[:], in_=xT_in[:])
            dst_sb = []
            v_sb = []
            idx_sb = []
            for t in (0, 1):
                d = sb.tile([128, nchs[t]], F32, tag=f"dst{t}", name=f"dst{t}")
                nc.sync.dma_start(out=d[:], in_=dst_in[t][:])
                dst_sb.append(d)
                vv = sb.tile([128, nchs[t]], F32, tag=f"v{t}", name=f"v{t}")
                nc.sync.dma_start(out=vv[:], in_=v_in[t][:])
                v_sb.append(vv)
                ii = sb.tile([128, nchs[t] * 8], I16, tag=f"idx{t}", name=f"idx{t}")
                nc.sync.dma_start(out=ii[:], in_=idx_in[t][:])
                idx_sb.append(ii)

            hT = sb.tile([128, SHARD], BF16, tag="hT")
            h1T = sb.tile([128, SHARD], BF16, tag="h1T")
            h2T = sb.tile([128, SHARD], F32, tag="h2T")

            # AllGather buffers (per layer, per etype)
            m_in = [[dr.tile([SHARD, 128], BF16, tag=f"mi{l}{t}", name=f"mi{l}{t}") for t in (0, 1)] for l in (0, 1)]
            m_out = [[dr.tile([VN, 128], BF16, tag=f"mo{l}{t}", name=f"mo{l}{t}", addr_space="Shared") for t in (0, 1)] for l in (0, 1)]

            qsem = [nc.alloc_semaphore(f"qsem{q}") for q in range(NQ)]
            qcnt = [0] * NQ

            def col_chunks(total, step):
                o = 0
                while o < total:
                    yield o, min(step, total - o)
                    o += step

            # ---- phase P: hT = (x @ W_proj + b_proj)^T
            for o, n in col_chunks(SHARD, 512):
                p = ps.tile([128, 512], F32, tag="pdense")
                nc.tensor.matmul(p[:, :n], lhsT=w_sb["W_proj"][:], rhs=xT[:, o : o + n],
                                 start=True, stop=True)
                nc.vector.tensor_scalar_add(hT[:, o : o + n], p[:, :n], b_sb["bias_proj"][:, :1])

            # ---- layers
            for l in (0, 1):
                src_hT = hT if l == 0 else h1T
                dstT = h1T if l == 0 else h2T
                wa, wb, wl = (("W1_a", "W1_b", "loop1") if l == 0 else ("W2_a", "W2_b", "loop2"))
                bias = b_sb["bias1"] if l == 0 else b_sb["bias2"]

                # m tables (node-major, bf16) + AllGather
                for t, wn in ((0, wa), (1, wb)):
                    for w in range(NW):
                        pm = ps.tile([128, 512], F32, tag="pdense")
                        nc.tensor.matmul(pm[:, :128], lhsT=src_hT[:, w * 128 : (w + 1) * 128],
                                         rhs=w_sb[wn][:], start=True, stop=True)
                        ms = sb.tile([128, 128], BF16, tag="ms")
                        nc.scalar.activation(out=ms[:], in_=pm[:, :128],
                                             func=mybir.ActivationFunctionType.Copy)
                        nc.sync.dma_start(out=m_in[l][t][w * 128 : (w + 1) * 128, :], in_=ms[:])
                    nc.gpsimd.collective_compute(
                        "AllGather",
                        mybir.AluOpType.bypass,
                        replica_groups=[list(range(NCORES))],
                        ins=[m_in[l][t].opt()],
                        outs=[m_out[l][t].opt()],
                    )

                # aggregation: per window, 4 gather pieces on 4 queues
                for w in range(NW):
                    pieces = [(t, hh) for t in (0, 1) for hh in (0, 1)]
                    gb = {}
                    for q, (t, hh) in enumerate(pieces):
                        kk = int(Ks[t][w, hh])
                        ci0 = offs[t][(w, hh)]
                        gbuf = sb.tile([128, kk, 128], BF16, tag=f"gb{t}{hh}",
                                       name=f"gb{l}{t}{hh}{w}", bufs=4)
                        nc.gpsimd.dma_gather(
                            gbuf[:],
                            m_out[l][t][hh * HALFR : (hh + 1) * HALFR, :],
                            idx_sb[t][:, ci0 * 8 : (ci0 + kk) * 8],
                            kk * 128,
                            kk * 128,
                            128,
                            single_packet=False,
                            prepare_only=True,
                            sem=qsem[q],
                            queue_num=q,
                        )
                        gb[(t, hh)] = gbuf
                        qcnt[q] += 1
                    for q in range(NQ):
                        nc.gpsimd.trigger_dma(count=None, queue_num=q)
                    for q in range(NQ):
                        nc.tensor.wait_ge(qsem[q], 16 * qcnt[q])

                    pw = ps.tile([128, 128], F32, tag="pw", bufs=4)
                    first = True
                    for t, hh in pieces:
                        gbuf = gb[(t, hh)]
                        for k in range(int(Ks[t][w, hh])):
                            ci = offs[t][(w, hh)] + k
                            # odd width + 2B-misaligned tile keeps this in
                            # DVE 1x mode (no shared-port lock vs SWDGE)
                            s = sb.tile([128, 130], BF16, tag="s", bufs=8)
                            nc.vector.tensor_scalar(
                                out=s[:, 1:130],
                                in0=iota_sb[:, 1:130],
                                scalar1=dst_sb[t][:, ci : ci + 1],
                                scalar2=v_sb[t][:, ci : ci + 1],
                                op0=mybir.AluOpType.is_equal,
                                op1=mybir.AluOpType.mult,
                            )
                            nc.tensor.matmul(pw[:], lhsT=gbuf[:, k, :], rhs=s[:, 1:129],
                                             start=first, stop=False)
                            first = False
                    nc.tensor.matmul(pw[:], lhsT=w_sb[wl][:],
                                     rhs=src_hT[:, w * 128 : (w + 1) * 128],
                                     start=False, stop=True)
                    nc.scalar.activation(out=dstT[:, w * 128 : (w + 1) * 128], in_=pw[:],
                                         func=mybir.ActivationFunctionType.Relu,
                                         bias=bias[:, :1], scale=1.0)

            nc.sync.dma_start(out=out[:], in_=h2T[:])
    nc.compile()
    return nc


def kernel(**inputs):
    x = np.asarray(inputs["x"], np.float32)
    prep_a = _prep_etype(np.asarray(inputs["src_a"]), np.asarray(inputs["dst_a"]))
    prep_b = _prep_etype(np.asarray(inputs["src_b"]), np.asarray(inputs["dst_b"]))
    K_a, off_a, nch_a, idx_a, dst_a, v_a = prep_a
    K_b, off_b, nch_b, idx_b, dst_b, v_b = prep_b

    key = (nch_a, nch_b, K_a.tobytes(), K_b.tobytes())
    if key not in _compiled:
        _compiled[key] = _build(K_a, off_a, nch_a, K_b, off_b, nch_b)
    nc = _compiled[key]

    x_pad = np.zeros((NCORES, SHARD, D), np.float32)
    x_pad[:, :LOCAL] = x.reshape(NCORES, LOCAL, D)

    weights = {
        "W_proj": inputs["W_proj"], "W1_a": inputs["W1_a"], "W1_b": inputs["W1_b"],
        "loop1": inputs["loop1"], "W2_a": inputs["W2_a"], "W2_b": inputs["W2_b"],
        "loop2": inputs["loop2"],
    }
    w_np = {k: np.asarray(v, np.float32).astype(BF) for k, v in weights.items()}
    biases = {
        "bias_proj": np.asarray(inputs["b_proj"], np.float32).reshape(128, 1),
        "bias1": (np.asarray(inputs["b1_a"], np.float32)
                  + np.asarray(inputs["b1_b"], np.float32)).reshape(128, 1),
        "bias2": (np.asarray(inputs["b2_a"], np.float32)
                  + np.asarray(inputs["b2_b"], np.float32)).reshape(128, 1),
    }
    # iota[:, j] = j-1 (cols 1..129 hold 0..128); col 0 unused
    iota = np.tile((np.arange(130, dtype=np.float32) - 1.0).astype(BF), (128, 1))

    in_maps = []
    for c in range(NCORES):
        m = {
            "xT": np.ascontiguousarray(x_pad[c].T).astype(BF),
            "iota": iota,
            "idx_a": idx_a[c], "idx_b": idx_b[c],
            "dst_a": dst_a[c], "dst_b": dst_b[c],
            "v_a": v_a[c], "v_b": v_b[c],
        }
        m.update(w_np)
        m.update(biases)
        in_maps.append(m)

    import os
    global last_results
    if os.environ.get("BASS_KERNEL_TRACE"):
        res = run_bass_kernel_spmd(nc, in_maps, core_ids=list(range(NCORES)),
                                   trace=True)
        last_results = res
    else:
        res = run_bass_kernel_spmd(nc, in_maps, core_ids=list(range(NCORES)))
    full = np.concatenate(
        [np.asarray(res.results[c]["out"]).T[:LOCAL] for c in range(NCORES)], axis=0
    )
    return full.astype(np.float32)
